# revision 1
# baseline (speedup 1.0000x reference)
"""CapsuleLayer (dynamic routing, 3 iters) Trainium2 Bass kernel.

Problem:
  inputs [B=64, In=2048, K=16] fp32, W [N=32, In=2048, D=32, K=16] fp32
  u_hat = einsum('nidk,bik->bnid'); 3 routing iterations; out v [B, N, D] fp32.

Strategy: pure data-parallel over B across 8 cores (W replicated), no
collectives. Per core (8 samples):
  Phase 1: einsum on PE with contraction dim (i8,k)=128 (8 input capsules
    packed per matmul via a host-built block-diagonal x operand). Output is
    produced directly transposed as u^T[(n4 d)=128, i] and spilled to DRAM
    scratch in fp16 (33.5 MB/core).
  Pass A: s1 = (1/32) sum_i u_hat  (c is uniform in iteration 1) via ACT
    copy-accumulate over the spilled tiles. v1 = squash(s1).
  Pass B/C (iterations 2,3): b_k = u . (v1+...+v_{k-1}) recomputed on the
    fly (b is a running sum of agreements, so only vsum is needed):
      t  = u . vsum        -> PE matmuls, lhsT = block-diag(vsum) [128,32]
      e  = exp(t)          -> ACT (no max subtraction; |t| < ~50, fp32 safe)
      Z  = sum_n e         -> PE ones-vector matmul (partition reduction)
      e' = e / Z           -> DVE, Z broadcast via GPSIMD partition_broadcast
      e_b = e' replicated over d -> PE matmul with constant selector E4x
      s += sum_i u * e_b   -> DVE tensor_tensor_reduce (fused mul+reduce)
    v_k = squash(s).  v3 is the output.

All shapes are hardcoded for this problem. Self-contained.
"""

import json
import os
import sys
from contextlib import ExitStack

import numpy as np

sys.path.insert(0, "/opt/trn_rl_repo")

import concourse.bass as bass  # noqa: E402
import concourse.mybir as mybir  # noqa: E402
import concourse.tile as tile  # noqa: E402

# ---------------------------------------------------------------------------
# Workaround for a walrus codegen crash in this container: Drain instructions
# that carry sem waits die in setupSyncWait<CTRL_NO_STRUCT>. Move each such
# wait onto a fresh EventSemaphore instruction inserted before the Drain.
# ---------------------------------------------------------------------------


def _fix_bir_json_bytes(raw: bytes) -> bytes:
    m = json.loads(raw)

    def blocks(b):
        if isinstance(b, dict):
            if "instructions" in b:
                yield b
            for v in b.values():
                yield from blocks(v)
        elif isinstance(b, list):
            for v in b:
                yield from blocks(v)

    no_fix = ("EventSemaphore", "RegisterMove", "UnconditionalBranch", "Call")
    for fn in m.get("functions", []):
        for bb in blocks(fn.get("blocks")):
            new_insts = []
            for inst in bb["instructions"]:
                si = inst.get("sync_info")
                if inst.get("opcode") not in no_fix and si and si.get("on_wait"):
                    waits = si["on_wait"]
                    # EventSemaphore supports at most 2 waits each
                    for wi in range(0, len(waits), 2):
                        new_insts.append(
                            {
                                "debug": inst.get("debug", 0),
                                "engine": inst["engine"],
                                "ins": [],
                                "name": f"{inst['name']}-waitfix{wi}",
                                "opcode": "EventSemaphore",
                                "outs": [],
                                "sync_info": {
                                    "on_update": [],
                                    "on_wait": waits[wi : wi + 2],
                                },
                            }
                        )
                    si["on_wait"] = []
                new_insts.append(inst)
            bb["instructions"] = new_insts
    return json.dumps(m).encode()


def _install_bir_fix():
    if getattr(bass.Bass, "_bir_fix_installed", False):
        return
    orig = bass.Bass.to_json_bytes

    def patched(self, *a, **k):
        return _fix_bir_json_bytes(orig(self, *a, **k))

    bass.Bass.to_json_bytes = patched
    bass.Bass._bir_fix_installed = True


_install_bir_fix()

# ---------------------------------------------------------------------------
# Problem constants
# ---------------------------------------------------------------------------
B, In, K = 64, 2048, 16
N, D = 32, 32
NCORES = 8
G = B // NCORES          # samples per core = 8
I8 = 8                   # input capsules packed per matmul chunk
NCHUNK = In // I8        # 256 chunks, contraction (i8,k) = 128
NDC = 8                  # nd-chunks: (n4 d) = 128 rows each, 8 of them
CG = 32                  # chunks per spill group (i-span = 256)
NGRP = NCHUNK // CG      # 8 spill groups
IB = 512                 # i-block width in routing passes
NIB = In // IB           # 4 i-blocks
EPS = 1e-7

FP32 = mybir.dt.float32
FP16 = mybir.dt.float16


def host_prep(inputs: np.ndarray, W: np.ndarray):
    """Build the per-core device operands (fp16).

    W_r [NCHUNK, 128, 1024] fp16 : W_r[c,(i8 k),(n d)] = W[n, 8c+i8, d, k]
    Xbd [NCHUNK, 128, 64] fp16 per core: block-diagonal x, cols (g, i8'),
        Xbd[c,(i8 k),(g i8')] = x[g, 8c+i8, k] * (i8 == i8')
    """
    W_r = (
        W.transpose(1, 3, 0, 2).reshape(NCHUNK, I8, K, N * D).reshape(NCHUNK, I8 * K, N * D)
    ).astype(np.float16)
    # batch 4 chunks per DMA: [NCHUNK/4, 128, 4, 1024]
    W_r4 = np.ascontiguousarray(W_r.reshape(NCHUNK // 4, 4, 128, N * D).transpose(0, 2, 1, 3))

    xbds = []
    for core in range(NCORES):
        x = inputs[core * G : (core + 1) * G]  # [G, In, K]
        xr = x.transpose(1, 2, 0).reshape(NCHUNK, I8, K, G)  # [c, i8, k, g]
        Z = np.zeros((NCHUNK, I8, K, G, I8), np.float16)
        idx = np.arange(I8)
        # Z[c, i8, k, g, i8] = xr[c, i8, k, g]
        Z[:, idx, :, :, idx] = xr.transpose(1, 0, 2, 3).astype(np.float16)
        xb = Z.reshape(NCHUNK, I8 * K, G * I8)
        # batch 16 chunks per DMA: [NCHUNK/16, 128, 16, 64]
        xbds.append(
            np.ascontiguousarray(xb.reshape(NCHUNK // 16, 16, 128, G * I8).transpose(0, 2, 1, 3))
        )
    return W_r4, xbds


def host_e4x() -> np.ndarray:
    """Constant selector: e4x[n, ng, (n4 d)] = (n == 4*ng + n4), fp16."""
    e = np.zeros((N, NDC, 128), np.float16)
    for ng in range(NDC):
        for n4 in range(4):
            e[4 * ng + n4, ng, 32 * n4 : 32 * n4 + 32] = 1.0
    return e


def build_bass(n_reps: int = 1) -> bass.Bass:
    """Emit the full per-core program. n_reps>1 wraps the body in a For_i
    hardware loop for timing runs."""
    nc = bass.Bass()
    w_r = nc.declare_dram_parameter("w_r", [NCHUNK // 4, 128, 4, N * D], FP16, isOutput=False)
    xbd = nc.declare_dram_parameter("xbd", [NCHUNK // 16, 128, 16, G * I8], FP16, isOutput=False)
    e4x_c = nc.declare_dram_parameter("e4x_c", [N, NDC, 128], FP16, isOutput=False)
    v_out = nc.declare_dram_parameter("v_out", [G, N, D], FP32, isOutput=True)

    with tile.TileContext(nc) as tc:
        with ExitStack() as ctx:
            if n_reps > 1:
                loop = ctx.enter_context(tc.For_i(0, n_reps, 1))
                del loop
            _emit_body(nc, tc, w_r, xbd, e4x_c, v_out)
    return nc


def _emit_body(nc, tc, w_r, xbd, e4x_c, v_out):
    with ExitStack() as ctx:
        _emit_body_inner(ctx, nc, tc, w_r, xbd, e4x_c, v_out)


def _emit_body_inner(ctx, nc, tc, w_r, xbd, e4x_c, v_out):
    # ---------------- persistent constants / buffers ----------------
    singles = ctx.enter_context(tc.tile_pool(name="singles", bufs=1))
    dram = ctx.enter_context(tc.tile_pool(name="dram", bufs=1, space="DRAM"))

    # u^T spill: single DRAM tensor [NDC, NIB, 128, G, IB] fp16; spills are
    # one DMA per (half, ndc) and reads one DMA per (g, ib) across all ndc
    u_spill = dram.tile([NDC, NIB, 128, G, IB], FP16, tag="u_spill", name="u_spill")

    # ones vector for the sum-over-n matmul: [N, 1] fp32
    ones32 = singles.tile([N, 1], FP32, tag="ones32")
    nc.vector.memset(ones32, 1.0)
    # ones row for the Z broadcast matmul: [1, N] fp32
    ones1xN = singles.tile([1, N], FP32, tag="ones1xN")
    nc.vector.memset(ones1xN, 1.0)

    # E4x[ng]: [N, 128] fp16 selector, E4x[ng][n, (n4 d)] = (n == 4*ng + n4)
    e4x = singles.tile([N, NDC, 128], FP16, tag="e4x")
    nc.sync.dma_start(out=e4x, in_=e4x_c[:])

    # Vbd32[(g, ng)]: [128, N] fp16 block-diag vsum, zeros persist
    vbd = singles.tile([128, G, NDC, N], FP16, tag="vbd")
    nc.vector.memset(vbd, 0.0)

    # squash / v state (fp32): sall [128, (g ndc)], sT/vT/vsumT [64, 128]
    sall = singles.tile([128, 128], FP32, tag="sall")
    nc.vector.memset(sall, 0.0)
    sT = singles.tile([G * NDC, 128], FP32, tag="sT")
    vsumT = singles.tile([G * NDC, 128], FP32, tag="vsumT")
    vT = singles.tile([G * NDC, 128], FP32, tag="vT")
    vsumT16 = singles.tile([G * NDC, 128], FP16, tag="vsumT16")
    vsum_dT = singles.tile([128, G * NDC], FP16, tag="vsum_dT")

    # ---------------- phase 1: einsum + spill + fused pass A ---------------
    # partsA[ndc]: [128, G, NIB] fp32 partial i-sums (pass A fused here)
    partsA = [
        singles.tile([128, G, NIB], FP32, tag=f"pa_{ndc}", name=f"pa_{ndc}")
        for ndc in range(NDC)
    ]
    CPH = In // NIB // I8 // 2  # chunk-pairs per ib-half = 32
    with ExitStack() as p1:
        wpool = p1.enter_context(tc.tile_pool(name="wtiles", bufs=3))
        xpool = p1.enter_context(tc.tile_pool(name="xtiles", bufs=3))
        stage_pool = p1.enter_context(tc.tile_pool(name="stage", bufs=2))
        psum1 = p1.enter_context(tc.tile_pool(name="psum1", bufs=3, space="PSUM"))

        for half in range(NIB):
            # stage[ndc]: [128, (g, i-span 512)] fp16
            stages = [
                stage_pool.tile([128, G, IB], FP16, tag=f"st_{ndc}", name=f"st_{ndc}_{half}")
                for ndc in range(NDC)
            ]
            w_t = None
            x_t = None
            for cpair in range(CPH):
                ptile = psum1.tile(
                    [128, 2, NDC, G * I8], FP32, tag="p1", name=f"p1_{half}_{cpair}"
                )
                for c2 in range(2):
                    c = half * 2 * CPH + cpair * 2 + c2
                    if c % 4 == 0:
                        w_t = wpool.tile([128, 4, N * D], FP16, tag="w")
                        nc.sync.dma_start(out=w_t, in_=w_r[c // 4])
                    if c % 16 == 0:
                        x_t = xpool.tile([128, 16, G * I8], FP16, tag="x")
                        nc.sync.dma_start(out=x_t, in_=xbd[c // 16])
                    for ndc in range(NDC):
                        # out[(n4 d), (g i8)] = W_slice^T @ Xbd
                        nc.tensor.matmul(
                            ptile[:, c2, ndc, :],
                            w_t[:, c % 4, 128 * ndc : 128 * (ndc + 1)],
                            x_t[:, c % 16, :],
                            start=True,
                            stop=True,
                        )
                for ndc in range(NDC):
                    # psum [128, (c2, g, i8)] -> stage free (g, c2, i8);
                    # split the copy stream between ACT and DVE
                    src = ptile[:, :, ndc, :]  # [128, 2, G*I8]
                    dst = stages[ndc][
                        :, :, cpair * 2 * I8 : (cpair * 2 + 2) * I8
                    ].rearrange("p g (c2 i8) -> p c2 g i8", c2=2)
                    if ndc % 2 == 0:
                        nc.vector.tensor_copy(
                            out=dst, in_=src.rearrange("p c2 (g i8) -> p c2 g i8", g=G)
                        )
                    else:
                        nc.scalar.copy(
                            out=dst, in_=src.rearrange("p c2 (g i8) -> p c2 g i8", g=G)
                        )
            for ndc in range(NDC):
                # fused pass A: partial sum over this half's i-span
                nc.vector.tensor_reduce(
                    out=partsA[ndc][:, :, half : half + 1].rearrange("p g one -> p (g one)"),
                    in_=stages[ndc],
                    axis=mybir.AxisListType.X,
                    op=mybir.AluOpType.add,
                )
                # one batched spill DMA per (half, ndc) via the ACT hwdge queue
                nc.scalar.dma_start(
                    out=u_spill[ndc, half],
                    in_=stages[ndc],
                )

    # ---------------- pass A epilogue: s1 = sum_i u / 32, v1 = squash(s1) --
    sall3 = sall[:, 0 : G * NDC].rearrange("p (g n) -> p g n", n=NDC)
    for ndc in range(NDC):
        nc.vector.tensor_reduce(
            out=sall3[:, :, ndc : ndc + 1].rearrange("p g one -> p (g one)"),
            in_=partsA[ndc],
            axis=mybir.AxisListType.X,
            op=mybir.AluOpType.add,
        )
    _emit_squash(nc, tc, sall, sT, vT, vsumT, scale_const=1.0 / 32.0, first=True)
    _emit_vsum_transpose(nc, tc, vsumT, vsumT16, vsum_dT)
    _emit_vbd_update(nc, tc, vsum_dT, vbd)

    # ---------------- passes B, C ----------------
    for pas in ("B", "C"):
        last = pas == "C"
        with ExitStack() as pb:
            upool = pb.enter_context(tc.tile_pool(name=f"u{pas}", bufs=4))
            epool = pb.enter_context(tc.tile_pool(name=f"e{pas}", bufs=6))
            spool = pb.enter_context(tc.tile_pool(name=f"s{pas}", bufs=3))
            psum_t = pb.enter_context(tc.tile_pool(name=f"pt{pas}", bufs=3, space="PSUM"))
            psum_z = pb.enter_context(tc.tile_pool(name=f"pz{pas}", bufs=2, space="PSUM"))
            psum_e = pb.enter_context(tc.tile_pool(name=f"pe{pas}", bufs=3, space="PSUM"))

            for g in range(G):
                s_parts = [
                    spool.tile([128, NIB], FP32, tag=f"sp{ng}", name=f"sp_{pas}_{g}_{ng}")
                    for ng in range(NDC)
                ]
                for ib in range(NIB):
                    # one batched read across all ndc groups
                    u_all = upool.tile(
                        [128, NDC, IB], FP16, tag="uall", name=f"ut_{pas}_{g}_{ib}"
                    )
                    qeng = nc.sync if (g * NIB + ib) % 2 == 0 else nc.scalar
                    qeng.dma_start(
                        out=u_all,
                        in_=u_spill[:, ib, :, g, :].rearrange("n p x -> p n x"),
                    )
                    u_ts = [u_all[:, ng, :] for ng in range(NDC)]
                    # t[n, i] accumulated over the 8 ngroups
                    t_ps = psum_t.tile([N, IB], FP32, tag="t")
                    for ng in range(NDC):
                        nc.tensor.matmul(
                            t_ps,
                            vbd[:, g, ng, :],
                            u_ts[ng],
                            start=(ng == 0),
                            stop=(ng == NDC - 1),
                        )
                    # e = exp(t)  [N, IB] fp32 in SBUF
                    e_t = epool.tile([N, IB], FP32, tag="e")
                    nc.scalar.activation(
                        out=e_t, in_=t_ps, func=mybir.ActivationFunctionType.Exp
                    )
                    # Z = sum_n e [1, IB] and its broadcast share one psum tile
                    zz_ps = psum_z.tile([2 * N, IB], FP32, tag="zz")
                    z_ps = zz_ps[0:1, :]
                    zrep_ps = zz_ps[N : 2 * N, :]
                    nc.tensor.matmul(z_ps, ones32, e_t, start=True, stop=True)
                    zinv = epool.tile([1, IB], FP32, tag="zinv")
                    nc.vector.reciprocal(out=zinv, in_=z_ps)
                    nc.tensor.matmul(zrep_ps, ones1xN, zinv, start=True, stop=True)
                    ep_t = epool.tile([N, IB], FP16, tag="ep")
                    nc.vector.tensor_mul(ep_t, e_t, zrep_ps)
                    # per ngroup: e_b = E4x[ng]^T @ e'  [128, IB] (broadcast
                    # over d), then s[ng] += sum_i u * e_b
                    for ng in range(NDC):
                        eb_ps = psum_e.tile([128, IB], FP32, tag="eb")
                        nc.tensor.matmul(
                            eb_ps, e4x[:, ng, :], ep_t, start=True, stop=True
                        )
                        eb_sb = epool.tile([128, IB], FP16, tag="ebs")
                        nc.scalar.copy(out=eb_sb, in_=eb_ps)
                        dummy = epool.tile([128, IB], FP16, tag="dum")
                        steng = nc.vector
                        steng.scalar_tensor_tensor(
                            out=dummy,
                            in0=u_ts[ng],
                            scalar=1.0,
                            in1=eb_sb,
                            op0=mybir.AluOpType.mult,
                            op1=mybir.AluOpType.mult,
                            accum_out=s_parts[ng][:, ib : ib + 1],
                        )
                for ng in range(NDC):
                    nc.vector.tensor_reduce(
                        out=sall[:, g * NDC + ng : g * NDC + ng + 1],
                        in_=s_parts[ng],
                        axis=mybir.AxisListType.X,
                        op=mybir.AluOpType.add,
                    )
            _emit_squash(nc, tc, sall, sT, vT, vsumT, scale_const=1.0, first=False)
            if last:
                # vT row (g,ndc), col (n4,d) -> v_out[g, 4*ndc+n4, d]; as a
                # flat [64, 4, 32] view with partition stride 128 this is
                # exactly v_out reshaped to [(g n/4), 4, 32].
                v_dst = v_out[:].rearrange("g n d -> (g n d)").rearrange(
                    "(p n4 d) -> p n4 d", n4=4, d=D
                )
                nc.sync.dma_start(out=v_dst, in_=vT)
            else:
                _emit_vsum_transpose(nc, tc, vsumT, vsumT16, vsum_dT)
                _emit_vbd_update(nc, tc, vsum_dT, vbd)


def _emit_squash(nc, tc, sall, sT, vT, vsumT, scale_const, first):
    """sall [128=(n4 d), 64=(g ndc)] -> vT [64, 128] fp32 = squash(s) rows,
    vsumT += vT."""
    # transpose to [64, 128] via fp16 (xbar transpose is 16-bit only)
    with ExitStack() as tctx:
        tp = tctx.enter_context(tc.tile_pool(name="tp", bufs=1))
        s16 = tp.tile([128, 128], FP16, tag="s16")
        nc.vector.tensor_copy(out=s16, in_=sall)
        sT16 = tp.tile([128, 128], FP16, tag="sT16")
        nc.sync.dma_start(out=sT16, in_=s16, transpose=True)
        nc.vector.tensor_copy(out=sT, in_=sT16[0 : G * NDC, :])
    if scale_const != 1.0:
        nc.vector.tensor_scalar_mul(sT, sT, scale_const)
    s3 = sT.rearrange("p (n4 d) -> p n4 d", n4=4)
    ssq_t = nc  # placeholder to keep names local
    # ssq [64, 4] = sum_d s^2
    with ExitStack() as ctx:
        tmp_pool = ctx.enter_context(tc.tile_pool(name="sq", bufs=1))
        sq = tmp_pool.tile([G * NDC, 128], FP32, tag="sq")
        nc.vector.tensor_mul(sq, sT, sT)
        ssq = tmp_pool.tile([G * NDC, 4], FP32, tag="ssq")
        nc.vector.tensor_reduce(
            out=ssq,
            in_=sq.rearrange("p (n4 d) -> p n4 d", n4=4),
            axis=mybir.AxisListType.X,
            op=mybir.AluOpType.add,
        )
        # scale = ssq / (1 + ssq) / sqrt(ssq + eps)
        onep = tmp_pool.tile([G * NDC, 4], FP32, tag="onep")
        nc.vector.tensor_scalar_add(onep, ssq, 1.0)
        rinv = tmp_pool.tile([G * NDC, 4], FP32, tag="rinv")
        nc.vector.reciprocal(out=rinv, in_=onep)
        epst = tmp_pool.tile([G * NDC, 1], FP32, tag="epst")
        nc.vector.memset(epst, float(EPS))
        rt = tmp_pool.tile([G * NDC, 4], FP32, tag="rt")
        nc.scalar.activation(
            out=rt,
            in_=ssq,
            func=mybir.ActivationFunctionType.Sqrt,
            bias=epst,
        )
        rtinv = tmp_pool.tile([G * NDC, 4], FP32, tag="rtinv")
        nc.vector.reciprocal(out=rtinv, in_=rt)
        scale = tmp_pool.tile([G * NDC, 4], FP32, tag="scale")
        nc.vector.tensor_mul(scale, ssq, rinv)
        nc.vector.tensor_mul(scale, scale, rtinv)
        # v = s * scale (broadcast over d via free-dim 0-step AP)
        scale_b = bass.AP(
            tensor=scale.tensor,
            offset=scale.offset,
            ap=[scale.ap[0], scale.ap[1], [0, 32]],
        )
        nc.vector.tensor_tensor(
            out=vT.rearrange("p (n4 d) -> p n4 d", n4=4),
            in0=s3,
            in1=scale_b,
            op=mybir.AluOpType.mult,
        )
        if first:
            nc.vector.tensor_copy(out=vsumT, in_=vT)
        else:
            nc.vector.tensor_add(vsumT, vsumT, vT)


def _emit_vsum_transpose(nc, tc, vsumT, vsumT16, vsum_dT):
    nc.vector.tensor_copy(out=vsumT16, in_=vsumT)
    nc.sync.dma_start(out=vsum_dT, in_=vsumT16, transpose=True)


def _emit_vbd_update(nc, tc, vsum_dT, vbd):
    """vsum_dT [128=(n4 d), 64=(g ndc)] -> vbd strips.
    vbd[32*n4:(n4+1)*32, g, ng, 4*ng+n4] = vsum_dT[32*n4:..., g*8+ng]"""
    for g in range(G):
        for ng in range(NDC):
            for n4 in range(4):
                nc.scalar.copy(
                    out=vbd[32 * n4 : 32 * n4 + 32, g, ng, 4 * ng + n4 : 4 * ng + n4 + 1],
                    in_=vsum_dT[32 * n4 : 32 * n4 + 32, g * NDC + ng : g * NDC + ng + 1],
                )


# ---------------------------------------------------------------------------
# entry point
# ---------------------------------------------------------------------------
_built = {}


def _get_nc(n_reps=1):
    if n_reps not in _built:
        _built[n_reps] = build_bass(n_reps)
    return _built[n_reps]


def kernel(inputs: np.ndarray, W: np.ndarray) -> np.ndarray:
    from concourse.bass_utils import run_bass_kernel_spmd

    inputs = np.asarray(inputs, np.float32)
    W = np.asarray(W, np.float32)
    W_r, xbds = host_prep(inputs, W)
    nc = _get_nc()
    e4x_np = host_e4x()
    in_maps = [{"w_r": W_r, "xbd": xbds[c], "e4x_c": e4x_np} for c in range(NCORES)]
    res = run_bass_kernel_spmd(nc, in_maps, core_ids=list(range(NCORES)))
    out = np.concatenate([res.results[c]["v_out"] for c in range(NCORES)], axis=0)
    return out.astype(np.float32)


if __name__ == "__main__":
    # tiny self-check of host_prep shapes
    x = np.random.randn(B, In, K).astype(np.float32)
    W = np.random.randn(N, In, D, K).astype(np.float32)
    W_r, xbds = host_prep(x, W)
    print(W_r.shape, xbds[0].shape)

